# revision 12
# baseline (speedup 1.0000x reference)
"""Trainium2 Bass kernel for nn_Attention_78554951844258.

Dense 12-head attention block: qkv = x@Wqkv+b; RoPE(q,k); softmax(q k^T/sqrt(d)) v; proj.

Sharding: data-parallel over batch — each of the 8 NeuronCores computes one
batch element end-to-end (no collectives).

Algebraic restructuring (host-side, exact, O(weights)):
  * The reference applies RoPE with seq_dim=1 on [b,h,n,d], so cos/sin depend
    only on (head, dim) — RoPE is a position-independent per-head 64x64 linear
    map M_h that folds into the q/k columns of w_qkv (and biases).
  * The softmax scale 1/sqrt(d) folds into the q weights.
  * The v bias and proj bias fold into a single output bias
    b_out = b_v @ w_proj + b_proj, because softmax rows sum to 1.
  * Softmax max-subtraction is skipped: folded scores are bounded (|S| < ~3),
    exp is safe in fp32 and the result is mathematically identical.

v2 schedule (HAM-aware): the v1 kernel ran ~360us because each head-pair's
normalization tail (6.5us single-partition DVE reciprocals feeding the next
pair's bias-adds through the in-order DVE queue) idled the PE >3.4us, HAM
re-throttled to 1.2GHz, and ~70% of the matmul stream ran at half clock.
v2 keeps the PE dense:
  * input DMAs ordered by consumption (xT, wv, wqk, biases, wp);
  * v_aug and the first q/k column-tiles pipeline through one 2-buffer PSUM
    tag before attention;
  * the remaining 10 q/k projections stream as PE filler inside the
    attention loop (their own 2-bank PSUM slot), hidden under the ACT-bound
    exp pipeline;
  * PSUM = pq(2 banks) + st(2) + pv(2x2) = 8 banks exactly;
  * normalization uses reciprocal_approx_fast (~1.3us, 18-bit) and gates
    nothing but the pair's own ovT; broadcast via DRAM round-trip DMA;
  * proj accumulates e=0..4 ahead of the last pair's e=5 chunks to shrink
    the tail.
Matmul operands are bf16; accumulation fp32 in PSUM.
"""
import numpy as np

NUM_HEADS = 12
E = 768
D = 64
B = 8
N = 1024
HALF = D // 2


def _ensure_axon_hooks():
    """The NTFF profile hook registry module may be missing in a fresh
    container; (re)create it so trace=True profiling degrades gracefully."""
    try:
        import antenv.axon_hooks  # noqa: F401
        return
    except ImportError:
        pass
    try:
        import antenv
        import os
        p = os.path.join(os.path.dirname(antenv.__file__), "axon_hooks.py")
        with open(p, "w") as f:
            f.write(
                "_hook = None\n\n"
                "def set_axon_ntff_profile_hook(hook):\n"
                "    global _hook\n    _hook = hook\n\n"
                "def get_axon_ntff_profile_hook():\n"
                "    return _hook\n")
    except Exception:
        pass


_ensure_axon_hooks()


# ---------------------------------------------------------------- host math
def _rope_matrix():
    """M[h, x, d]: rope(q)[x] = sum_d M[h, x, d] * q[d] (float64)."""
    inv_freq = 1.0 / (10000.0 ** (np.arange(0, D, 2, dtype=np.float64) / D))
    t = np.arange(NUM_HEADS, dtype=np.float64)
    emb = np.concatenate([t[:, None] * inv_freq[None, :]] * 2, axis=-1)  # [H, D]
    cos, sin = np.cos(emb), np.sin(emb)
    M = np.zeros((NUM_HEADS, D, D))
    for h in range(NUM_HEADS):
        for d in range(D):
            M[h, d, d] = cos[h, d]
            if d < HALF:
                M[h, d, d + HALF] = -sin[h, d]
            else:
                M[h, d, d - HALF] = sin[h, d]
    return M


def _prep_weights(w_qkv, b_qkv, w_proj, b_proj):
    w = w_qkv.astype(np.float64)
    b = b_qkv.astype(np.float64)
    M = _rope_matrix()
    scale = float(D) ** (-0.5)
    w_q = w[:, 0:E].reshape(E, NUM_HEADS, D)
    w_k = w[:, E:2 * E].reshape(E, NUM_HEADS, D)
    b_q = b[0:E].reshape(NUM_HEADS, D)
    b_k = b[E:2 * E].reshape(NUM_HEADS, D)
    w_q2 = np.einsum('ehd,hxd->ehx', w_q, M) * scale
    b_q2 = np.einsum('hd,hxd->hx', b_q, M) * scale
    w_k2 = np.einsum('ehd,hxd->ehx', w_k, M)
    b_k2 = np.einsum('hd,hxd->hx', b_k, M)
    w_qk = np.ascontiguousarray(
        np.concatenate([w_q2.reshape(E, E), w_k2.reshape(E, E)], axis=1),
        dtype=np.float32)                                     # [E, 2E]
    b_qk = np.concatenate([b_q2.reshape(E), b_k2.reshape(E)]).astype(np.float32)
    w_v = np.ascontiguousarray(w[:, 2 * E:3 * E], dtype=np.float32)
    b_out = (b[2 * E:3 * E] @ w_proj.astype(np.float64)
             + b_proj.astype(np.float64)).astype(np.float32)
    return w_qk, b_qk, w_v, b_out


# ---------------------------------------------------------------- waitfix
def _split_excess_waits(nc):
    """walrus in this container rejects >4 sync waits per instruction (and
    fewer on Drain/SP-NoOp paths). Split overflow waits onto preceding
    same-engine 1-wait NOPs — semantically identical (sequencer blocks in
    order)."""
    import concourse.mybir as mybir
    import bass_rust
    counter = [0]

    def make_nop(engine):
        counter[0] += 1
        nop = bass_rust.InstNoOp(name=f"I-waitfix-{counter[0]}", ins=[], outs=[])
        nop.engine = engine
        return nop

    for fn in nc.m.functions:
        for bb in fn.blocks:
            insts = bb.instructions
            out = []
            changed = False
            for inst in insts:
                si = inst.sync_info
                waits = list(si.on_wait) if si is not None else []
                tn = type(inst).__name__
                keep = 0 if tn == "InstDrain" else 1
                if len(waits) > keep:
                    for w in waits[:len(waits) - keep]:
                        nop = make_nop(inst.engine)
                        nop.sync_info = mybir.SyncInfo(on_wait=[w], on_update=[])
                        out.append(nop)
                    inst.sync_info = mybir.SyncInfo(
                        on_wait=waits[len(waits) - keep:],
                        on_update=list(si.on_update))
                    changed = True
                out.append(inst)
            if changed:
                bb.instructions = out


# ---------------------------------------------------------------- device IR
_NC_CACHE = []


def _build_nc():
    import concourse.bass as bass
    import concourse.mybir as mybir
    from concourse.tile import TileContext

    dt = mybir.dt
    f32 = dt.float32
    bf16 = dt.bfloat16
    AF = mybir.ActivationFunctionType

    nc = bass.Bass(target_bir_lowering=False)
    xT_d = nc.dram_tensor("xT", [E, N], bf16, kind="ExternalInput")
    wqk_d = nc.dram_tensor("w_qk", [E, 2 * E], bf16, kind="ExternalInput")
    bqk_d = nc.dram_tensor("b_qk", [2 * E], f32, kind="ExternalInput")
    wv_d = nc.dram_tensor("w_v", [E, E], bf16, kind="ExternalInput")
    wp_d = nc.dram_tensor("w_proj", [E, E], bf16, kind="ExternalInput")
    bo_d = nc.dram_tensor("b_out", [E], f32, kind="ExternalInput")
    y_d = nc.dram_tensor("y", [N, E], f32, kind="ExternalOutput")

    ET = E // 128          # 6 e-tiles
    IT = N // 128          # 8 i/j-tiles
    HP = NUM_HEADS // 2    # 6 head pairs

    with TileContext(nc) as tc:
        with (
            tc.tile_pool(name="stat", bufs=1) as p1,         # xT, w_qk, wv, wp
            tc.tile_pool(name="persist", bufs=1) as pp,      # v_aug, qkt, ovT, biases
            tc.tile_pool(name="pT", bufs=4) as ppT,          # exp'd scores
            tc.tile_pool(name="nrm", bufs=6) as prb,         # recip + broadcast
            tc.tile_pool(name="yout", bufs=2) as py,         # y staging
            tc.tile_pool(name="dscr", bufs=4, space="DRAM") as pdram,
        ):
            # ---- loads, ordered by first use
            xT = [p1.tile([128, N], bf16, tag=f"xT{e}", name=f"xT{e}")
                  for e in range(ET)]
            wv = [p1.tile([128, E], bf16, tag=f"wv{e}", name=f"wv{e}")
                  for e in range(ET)]
            wqk = [p1.tile([128, 2 * E], bf16, tag=f"wqk{e}", name=f"wqk{e}")
                   for e in range(ET)]
            wp = [p1.tile([128, E], bf16, tag=f"wp{e}", name=f"wp{e}")
                  for e in range(ET)]
            for e in range(ET):
                nc.sync.dma_start(out=xT[e], in_=xT_d[e * 128:(e + 1) * 128, :])
            for e in range(ET):
                nc.sync.dma_start(out=wv[e], in_=wv_d[e * 128:(e + 1) * 128, :])
            for e in range(ET):
                nc.sync.dma_start(out=wqk[e], in_=wqk_d[e * 128:(e + 1) * 128, :])
            bq = pp.tile([128, 12], f32, tag="bq")
            nc.sync.dma_start(out=bq, in_=bqk_d[:].rearrange("(t p) -> p t", p=128))
            bo = pp.tile([128, E], f32, tag="bo")
            nc.sync.dma_start(
                out=bo,
                in_=bass.AP(tensor=bo_d[:].tensor, offset=bo_d[:].offset,
                            ap=[[0, 128], [1, E]]))
            for e in range(ET):
                nc.sync.dma_start(out=wp[e], in_=wp_d[e * 128:(e + 1) * 128, :])

            v_aug = [pp.tile([128, NUM_HEADS * (D + 1)], bf16, tag=f"vaug{i}",
                             name=f"vaug{i}") for i in range(IT)]
            qkt = [pp.tile([128, N], bf16, tag=f"qkt{c}", name=f"qkt{c}")
                   for c in range(2 * ET)]

            # q/k column-tile projection: 12 accumulating MMs + bias-add.
            # Emitted eagerly for ct 0/6, then as PE filler inside attention.
            def emit_qk_mm(pq, ct, i):
                ih, e = divmod(i, ET)
                nc.tensor.matmul(
                    pq[:, ih * 512:(ih + 1) * 512],
                    wqk[e][:, ct * 128:(ct + 1) * 128],
                    xT[e][:, ih * 512:(ih + 1) * 512],
                    start=(e == 0), stop=(e == ET - 1))

            def emit_qk_bias(pq, ct):
                nc.vector.tensor_scalar_add(qkt[ct], pq, bq[:, ct:ct + 1])

            # ---- pre-attention: v_aug (8 tiles) + all 12 q/k column tiles
            # through one 2-buffer psum tag (scoped pool; space reclaimed
            # after). v and qk interleave so the DVE work (one strided cast
            # per v tile, one bias-add per qk tile) stays off the MM rhythm.
            with tc.tile_pool(name="psA", bufs=2, space="PSUM") as psA:
                # exact 1.0 into the per-head ones columns (DVE in0*0 + 1;
                # strided memset is rejected by this walrus's ISA check) —
                # independent of the matmuls, emitted up front
                bq12 = bq[:, 0:12].rearrange("p (a b) -> p a b", b=1)
                for it in range(IT):
                    ones_cols = v_aug[it].rearrange(
                        "p (h c) -> p h c", c=65)[:, :, 64:65]
                    nc.vector.tensor_scalar(
                        ones_cols, bq12, 0.0, 1.0,
                        mybir.AluOpType.mult, mybir.AluOpType.add)

                def emit_v(it):
                    pvv = psA.tile([128, N], f32, tag="vq", name=f"pv_{it}")
                    for (n0, nw) in ((0, 512), (512, 256)):
                        for e in range(ET):
                            nc.tensor.matmul(
                                pvv[:, n0:n0 + nw],
                                xT[e][:, it * 128:(it + 1) * 128],
                                wv[e][:, n0:n0 + nw],
                                start=(e == 0), stop=(e == ET - 1))
                    # single strided cast: [128,768] f32 -> per-head 64-col
                    # groups of v_aug (stride 65)
                    nc.vector.tensor_copy(
                        out=v_aug[it].rearrange(
                            "p (h c) -> p h c", c=65)[:, :, 0:64],
                        in_=pvv[:, 0:E].rearrange("p (h c) -> p h c", c=64))

                def emit_qk(ct):
                    pq = psA.tile([128, N], f32, tag="vq", name=f"pq_{ct}")
                    for i in range(12):
                        emit_qk_mm(pq, ct, i)
                    emit_qk_bias(pq, ct)

                # attention pair hp needs cts (hp, 6+hp); order qk so pair 0
                # unblocks first, v tiles spread between
                qk_order = [0, ET, 1, ET + 1, 2, ET + 2, 3, ET + 3,
                            4, ET + 4, 5, ET + 5]
                emit_v(0)
                for z in range(8):
                    emit_qk(qk_order[z])
                    if z + 1 < IT:
                        emit_v(z + 1)
                for z in range(8, 12):
                    emit_qk(qk_order[z])

            # ---- attention: per head-pair; next pair's q/k projections
            # interleave as PE filler (3 MMs per jt slot)
            ovT = [pp.tile([128, N], bf16, tag=f"ovT{e}", name=f"ovT{e}")
                   for e in range(ET)]

            with tc.tile_pool(name="psB", bufs=1, space="PSUM") as psB:
                for hp in range(HP):
                    h0, h1 = 2 * hp, 2 * hp + 1
                    qt, kt = qkt[hp], qkt[ET + hp]
                    pv0 = psB.tile([65, N], f32, tag="pv", bufs=2,
                                   name=f"pv0_{hp}")
                    pv1 = psB.tile([65, N], f32, tag="pv", bufs=2,
                                   name=f"pv1_{hp}")
                    for jt in range(IT):
                        js = slice(jt * 128, (jt + 1) * 128)
                        st0 = psB.tile([128, N], f32, tag="st", bufs=2,
                                       name=f"st0_{hp}_{jt}")
                        for ih in range(2):
                            isl = slice(ih * 512, (ih + 1) * 512)
                            nc.tensor.matmul(st0[:, isl], kt[0:64, js],
                                             qt[0:64, isl])
                        pT0 = ppT.tile([128, N], bf16, tag="pT",
                                       name=f"pT0_{hp}_{jt}")
                        nc.scalar.activation(out=pT0, in_=st0, func=AF.Exp)
                        st1 = psB.tile([128, N], f32, tag="st", bufs=2,
                                       name=f"st1_{hp}_{jt}")
                        for ih in range(2):
                            isl = slice(ih * 512, (ih + 1) * 512)
                            nc.tensor.matmul(st1[:, isl], kt[64:128, js],
                                             qt[64:128, isl])
                        pT1 = ppT.tile([128, N], bf16, tag="pT",
                                       name=f"pT1_{hp}_{jt}")
                        nc.scalar.activation(out=pT1, in_=st1, func=AF.Exp)
                        for ih in range(2):
                            isl = slice(ih * 512, (ih + 1) * 512)
                            nc.tensor.matmul(
                                pv0[:, isl],
                                v_aug[jt][:, h0 * 65:h0 * 65 + 65],
                                pT0[:, isl], start=(jt == 0),
                                stop=(jt == IT - 1))
                        for ih in range(2):
                            isl = slice(ih * 512, (ih + 1) * 512)
                            nc.tensor.matmul(
                                pv1[:, isl],
                                v_aug[jt][:, h1 * 65:h1 * 65 + 65],
                                pT1[:, isl], start=(jt == 0),
                                stop=(jt == IT - 1))

                    # normalization. First evict both accumulators to SBUF
                    # with two fast DVE copies — this frees the pv PSUM
                    # slots ~2us after the pair ends so the next pair's PV
                    # matmuls never stall (the full normalization chain is
                    # ~8us of serial DVE+DMA latency and would otherwise
                    # idle the PE long enough to re-throttle HAM every
                    # pair). Then: colsum rows round-trip through DRAM
                    # reshaped to [128,16] so the multi-cycle DVE reciprocal
                    # runs on 128 lanes (~0.3us, not 6.5us); DRAM-broadcast
                    # 1/s to 64 partitions (SBUF APs cannot have zero
                    # partition step, DRAM APs can); multiply. Odd head
                    # moved into its ovT partition range by DMA (DVE cannot
                    # cross partitions).
                    s0 = prb.tile([65, N], f32, tag="pvc", bufs=4,
                                  name=f"s0_{hp}")
                    s1 = prb.tile([65, N], f32, tag="pvc", bufs=4,
                                  name=f"s1_{hp}")
                    nc.vector.tensor_copy(out=s0, in_=pv0)
                    nc.vector.tensor_copy(out=s1, in_=pv1)
                    ds = pdram.tile([2, N], f32, tag="ds", name=f"ds_{hp}")
                    nc.sync.dma_start(out=ds[0:1, :], in_=s0[64:65, :])
                    nc.sync.dma_start(out=ds[1:2, :], in_=s1[64:65, :])
                    rsq = prb.tile([128, 16], f32, tag="rsq", name=f"rsq_{hp}")
                    nc.sync.dma_start(
                        out=rsq,
                        in_=bass.AP(tensor=ds.tensor, offset=ds.offset,
                                    ap=[[16, 128], [1, 16]]))
                    rrq = prb.tile([128, 16], f32, tag="rsq", name=f"rrq_{hp}")
                    nc.vector.reciprocal(out=rrq, in_=rsq)
                    dr = pdram.tile([2, N], f32, tag="dr", name=f"dr_{hp}")
                    nc.sync.dma_start(
                        out=bass.AP(tensor=dr.tensor, offset=dr.offset,
                                    ap=[[16, 128], [1, 16]]),
                        in_=rrq)
                    rb0 = prb.tile([64, N], f32, tag="nrm", name=f"rb0_{hp}")
                    rb1 = prb.tile([64, N], f32, tag="nrm", name=f"rb1_{hp}")
                    nc.sync.dma_start(
                        out=rb0,
                        in_=bass.AP(tensor=dr.tensor, offset=dr.offset,
                                    ap=[[0, 64], [1, N]]))
                    nc.sync.dma_start(
                        out=rb1,
                        in_=bass.AP(tensor=dr.tensor, offset=dr[1:2, :].offset,
                                    ap=[[0, 64], [1, N]]))
                    nc.vector.tensor_mul(ovT[hp][0:64, :], s0[0:64, :], rb0)
                    tmp1 = prb.tile([64, N], bf16, tag="nrm", name=f"tmp1_{hp}")
                    nc.vector.tensor_mul(tmp1, s1[0:64, :], rb1)
                    nc.sync.dma_start(out=ovT[hp][64:128, :], in_=tmp1)

                # ---- proj: y = ovT^T @ w_proj + b_out. Lives inside the
                # attention pool reusing the 'st' slots (no pool-boundary
                # drain). e=0..4 for an it-pair emit before their e=5 chunks
                # so only the last head-pair's ovT gates the tail.
                for g in range(IT // 2):
                    its = (2 * g, 2 * g + 1)
                    pyts = {}
                    for it in its:
                        isl = slice(it * 128, (it + 1) * 128)
                        pyt = psB.tile([128, E], f32, tag="st", bufs=2,
                                       name=f"py_{it}")
                        pyts[it] = pyt
                        for (n0, nw) in ((0, 512), (512, 256)):
                            for e in range(ET - 1):
                                nc.tensor.matmul(
                                    pyt[:, n0:n0 + nw],
                                    ovT[e][:, isl],
                                    wp[e][:, n0:n0 + nw],
                                    start=(e == 0), stop=False)
                    for it in its:
                        isl = slice(it * 128, (it + 1) * 128)
                        pyt = pyts[it]
                        for (n0, nw) in ((0, 512), (512, 256)):
                            nc.tensor.matmul(
                                pyt[:, n0:n0 + nw],
                                ovT[ET - 1][:, isl],
                                wp[ET - 1][:, n0:n0 + nw],
                                start=False, stop=True)
                        ysb = py.tile([128, E], f32, tag="y", name=f"y{it}")
                        nc.vector.tensor_add(ysb, pyt, bo)
                        nc.sync.dma_start(out=y_d[isl, :], in_=ysb)

    _split_excess_waits(nc)
    return nc


def _get_nc():
    if not _NC_CACHE:
        _NC_CACHE.append(_build_nc())
    return _NC_CACHE[0]


# ---------------------------------------------------------------- entry point
def kernel(x, w_qkv, b_qkv, w_proj, b_proj, _trace=False):
    from concourse.bass_utils import run_bass_kernel_spmd

    import ml_dtypes
    bf16 = ml_dtypes.bfloat16
    x = np.asarray(x)
    w_qk, b_qk, w_v, b_out = _prep_weights(
        np.asarray(w_qkv), np.asarray(b_qkv), np.asarray(w_proj),
        np.asarray(b_proj))
    w_qk16 = w_qk.astype(bf16)
    w_v16 = w_v.astype(bf16)
    w_proj16 = np.ascontiguousarray(np.asarray(w_proj)).astype(bf16)

    in_maps = []
    for b in range(B):
        in_maps.append({
            "xT": np.ascontiguousarray(x[b].T).astype(bf16),
            "w_qk": w_qk16,
            "b_qk": b_qk,
            "w_v": w_v16,
            "w_proj": w_proj16,
            "b_out": b_out,
        })

    nc = _get_nc()
    res = run_bass_kernel_spmd(nc, in_maps, core_ids=list(range(B)),
                               trace=_trace)
    out = np.stack([res.results[b]["y"] for b in range(B)]).astype(np.float32)
    if _trace:
        return out, res
    return out


# revision 18
# speedup vs baseline: 1.0666x; 1.0666x over previous
"""Trainium2 Bass kernel for nn_Attention_78554951844258.

Dense 12-head attention block: qkv = x@Wqkv+b; RoPE(q,k); softmax(q k^T/sqrt(d)) v; proj.

Sharding: data-parallel over batch — each of the 8 NeuronCores computes one
batch element end-to-end (no collectives).

Algebraic restructuring (host-side, exact, O(weights)):
  * The reference applies RoPE with seq_dim=1 on [b,h,n,d], so cos/sin depend
    only on (head, dim) — RoPE is a position-independent per-head 64x64 linear
    map M_h that folds into the q/k columns of w_qkv (and biases).
  * The softmax scale 1/sqrt(d) folds into the q weights.
  * The v bias and proj bias fold into a single output bias
    b_out = b_v @ w_proj + b_proj, because softmax rows sum to 1.
  * Softmax max-subtraction is skipped: folded scores are bounded (|S| < ~3),
    exp is safe in fp32 and the result is mathematically identical.

v2 schedule (HAM-aware): the v1 kernel ran ~360us because each head-pair's
normalization tail (6.5us single-partition DVE reciprocals feeding the next
pair's bias-adds through the in-order DVE queue) idled the PE >3.4us, HAM
re-throttled to 1.2GHz, and ~70% of the matmul stream ran at half clock.
v2 keeps the PE dense:
  * input DMAs ordered by consumption (xT, wv, wqk, biases, wp);
  * v_aug and the first q/k column-tiles pipeline through one 2-buffer PSUM
    tag before attention;
  * the remaining 10 q/k projections stream as PE filler inside the
    attention loop (their own 2-bank PSUM slot), hidden under the ACT-bound
    exp pipeline;
  * PSUM = pq(2 banks) + st(2) + pv(2x2) = 8 banks exactly;
  * normalization uses reciprocal_approx_fast (~1.3us, 18-bit) and gates
    nothing but the pair's own ovT; broadcast via DRAM round-trip DMA;
  * proj accumulates e=0..4 ahead of the last pair's e=5 chunks to shrink
    the tail.
Matmul operands are bf16; accumulation fp32 in PSUM.
"""
import numpy as np

NUM_HEADS = 12
E = 768
D = 64
B = 8
N = 1024
HALF = D // 2


def _ensure_axon_hooks():
    """The NTFF profile hook registry module may be missing in a fresh
    container; (re)create it so trace=True profiling degrades gracefully."""
    try:
        import antenv.axon_hooks  # noqa: F401
        return
    except ImportError:
        pass
    try:
        import antenv
        import os
        p = os.path.join(os.path.dirname(antenv.__file__), "axon_hooks.py")
        with open(p, "w") as f:
            f.write(
                "_hook = None\n\n"
                "def set_axon_ntff_profile_hook(hook):\n"
                "    global _hook\n    _hook = hook\n\n"
                "def get_axon_ntff_profile_hook():\n"
                "    return _hook\n")
    except Exception:
        pass


_ensure_axon_hooks()


# ---------------------------------------------------------------- host math
def _rope_matrix():
    """M[h, x, d]: rope(q)[x] = sum_d M[h, x, d] * q[d] (float64)."""
    inv_freq = 1.0 / (10000.0 ** (np.arange(0, D, 2, dtype=np.float64) / D))
    t = np.arange(NUM_HEADS, dtype=np.float64)
    emb = np.concatenate([t[:, None] * inv_freq[None, :]] * 2, axis=-1)  # [H, D]
    cos, sin = np.cos(emb), np.sin(emb)
    M = np.zeros((NUM_HEADS, D, D))
    for h in range(NUM_HEADS):
        for d in range(D):
            M[h, d, d] = cos[h, d]
            if d < HALF:
                M[h, d, d + HALF] = -sin[h, d]
            else:
                M[h, d, d - HALF] = sin[h, d]
    return M


def _prep_weights(w_qkv, b_qkv, w_proj, b_proj):
    w = w_qkv.astype(np.float64)
    b = b_qkv.astype(np.float64)
    M = _rope_matrix()
    scale = float(D) ** (-0.5)
    w_q = w[:, 0:E].reshape(E, NUM_HEADS, D)
    w_k = w[:, E:2 * E].reshape(E, NUM_HEADS, D)
    b_q = b[0:E].reshape(NUM_HEADS, D)
    b_k = b[E:2 * E].reshape(NUM_HEADS, D)
    w_q2 = np.einsum('ehd,hxd->ehx', w_q, M) * scale
    b_q2 = np.einsum('hd,hxd->hx', b_q, M) * scale
    w_k2 = np.einsum('ehd,hxd->ehx', w_k, M)
    b_k2 = np.einsum('hd,hxd->hx', b_k, M)
    w_qk = np.ascontiguousarray(
        np.concatenate([w_q2.reshape(E, E), w_k2.reshape(E, E)], axis=1),
        dtype=np.float32)                                     # [E, 2E]
    b_qk = np.concatenate([b_q2.reshape(E), b_k2.reshape(E)]).astype(np.float32)
    w_v = np.ascontiguousarray(w[:, 2 * E:3 * E], dtype=np.float32)
    b_out = (b[2 * E:3 * E] @ w_proj.astype(np.float64)
             + b_proj.astype(np.float64)).astype(np.float32)
    return w_qk, b_qk, w_v, b_out


# ---------------------------------------------------------------- waitfix
def _split_excess_waits(nc):
    """walrus in this container rejects >4 sync waits per instruction (and
    fewer on Drain/SP-NoOp paths). Split overflow waits onto preceding
    same-engine 1-wait NOPs — semantically identical (sequencer blocks in
    order)."""
    import concourse.mybir as mybir
    import bass_rust
    counter = [0]

    def make_nop(engine):
        counter[0] += 1
        nop = bass_rust.InstNoOp(name=f"I-waitfix-{counter[0]}", ins=[], outs=[])
        nop.engine = engine
        return nop

    for fn in nc.m.functions:
        for bb in fn.blocks:
            insts = bb.instructions
            out = []
            changed = False
            for inst in insts:
                si = inst.sync_info
                waits = list(si.on_wait) if si is not None else []
                tn = type(inst).__name__
                keep = 0 if tn == "InstDrain" else 1
                if len(waits) > keep:
                    for w in waits[:len(waits) - keep]:
                        nop = make_nop(inst.engine)
                        nop.sync_info = mybir.SyncInfo(on_wait=[w], on_update=[])
                        out.append(nop)
                    inst.sync_info = mybir.SyncInfo(
                        on_wait=waits[len(waits) - keep:],
                        on_update=list(si.on_update))
                    changed = True
                out.append(inst)
            if changed:
                bb.instructions = out


# ---------------------------------------------------------------- device IR
_NC_CACHE = []


def _build_nc():
    import concourse.bass as bass
    import concourse.mybir as mybir
    from concourse.tile import TileContext

    dt = mybir.dt
    f32 = dt.float32
    bf16 = dt.bfloat16
    AF = mybir.ActivationFunctionType

    nc = bass.Bass(target_bir_lowering=False)
    xT_d = nc.dram_tensor("xT", [E, N], bf16, kind="ExternalInput")
    wqk_d = nc.dram_tensor("w_qk", [E, 2 * E], bf16, kind="ExternalInput")
    bqk_d = nc.dram_tensor("b_qk", [2 * E], f32, kind="ExternalInput")
    wv_d = nc.dram_tensor("w_v", [E, E], bf16, kind="ExternalInput")
    wp_d = nc.dram_tensor("w_proj", [E, E], bf16, kind="ExternalInput")
    bo_d = nc.dram_tensor("b_out", [E], f32, kind="ExternalInput")
    y_d = nc.dram_tensor("y", [N, E], f32, kind="ExternalOutput")

    ET = E // 128          # 6 e-tiles
    IT = N // 128          # 8 i/j-tiles
    HP = NUM_HEADS // 2    # 6 head pairs

    with TileContext(nc) as tc:
        with (
            tc.tile_pool(name="stat", bufs=1) as p1,         # xT, w_qk, wv, wp
            tc.tile_pool(name="persist", bufs=1) as pp,      # v_aug, qkt, ovT, biases
            tc.tile_pool(name="pT", bufs=4) as ppT,          # exp'd scores
            tc.tile_pool(name="nrm", bufs=6) as prb,         # recip + broadcast
            tc.tile_pool(name="yout", bufs=2) as py,         # y staging
            tc.tile_pool(name="dscr", bufs=4, space="DRAM") as pdram,
        ):
            # ---- loads, ordered by first use
            xT = [p1.tile([128, N], bf16, tag=f"xT{e}", name=f"xT{e}")
                  for e in range(ET)]
            wv = [p1.tile([128, E], bf16, tag=f"wv{e}", name=f"wv{e}")
                  for e in range(ET)]
            wqk = [p1.tile([128, 2 * E], bf16, tag=f"wqk{e}", name=f"wqk{e}")
                   for e in range(ET)]
            wp = [p1.tile([128, E], bf16, tag=f"wp{e}", name=f"wp{e}")
                  for e in range(ET)]
            for e in range(ET):
                nc.sync.dma_start(out=xT[e], in_=xT_d[e * 128:(e + 1) * 128, :])
            for e in range(ET):
                nc.sync.dma_start(out=wv[e], in_=wv_d[e * 128:(e + 1) * 128, :])
            for e in range(ET):
                nc.sync.dma_start(out=wqk[e], in_=wqk_d[e * 128:(e + 1) * 128, :])
            bq = pp.tile([128, 12], f32, tag="bq")
            nc.sync.dma_start(out=bq, in_=bqk_d[:].rearrange("(t p) -> p t", p=128))
            bo = pp.tile([128, E], f32, tag="bo")
            nc.sync.dma_start(
                out=bo,
                in_=bass.AP(tensor=bo_d[:].tensor, offset=bo_d[:].offset,
                            ap=[[0, 128], [1, E]]))
            for e in range(ET):
                nc.sync.dma_start(out=wp[e], in_=wp_d[e * 128:(e + 1) * 128, :])

            # v_aug: per head 128 columns (64 v + ones + 63 zeros) so the PV
            # stationary is a full 128x128 load — HAM's activity monitor
            # only un-throttles the PE clock when the whole array is lit;
            # half-idle matmuls (M=65 / K=64) keep it at 1.2 GHz forever.
            v_aug = [pp.tile([128, NUM_HEADS * 128], bf16, tag=f"vaug{i}",
                             name=f"vaug{i}") for i in range(IT)]
            # q is stored zero-padded per head ([128, N] with the other
            # head's 64 partitions zeroed) so the S matmul contracts K=128
            # against the full shared kt stationary; k tiles stay packed.
            qtp = [[pp.tile([128, N], bf16, tag=f"qtp{c}_{h}",
                            name=f"qtp{c}_{h}") for h in range(2)]
                   for c in range(ET)]
            ktt = [pp.tile([128, N], bf16, tag=f"ktt{c}", name=f"ktt{c}")
                   for c in range(ET)]
            # zero-fills: no input deps — run during the load phase
            for it in range(IT):
                nc.vector.memset(v_aug[it], 0.0)
            for c in range(ET):
                nc.vector.memset(qtp[c][0][64:128, :], 0.0)
                nc.vector.memset(qtp[c][1][0:64, :], 0.0)

            # q/k column-tile projection: 12 accumulating MMs + bias-add
            def emit_qk_mm(pq, ct, i):
                ih, e = divmod(i, ET)
                nc.tensor.matmul(
                    pq[:, ih * 512:(ih + 1) * 512],
                    wqk[e][:, ct * 128:(ct + 1) * 128],
                    xT[e][:, ih * 512:(ih + 1) * 512],
                    start=(e == 0), stop=(e == ET - 1))

            def emit_qk_bias(pq, ct):
                if ct < ET:
                    nc.vector.tensor_scalar_add(
                        qtp[ct][0][0:64, :], pq[0:64, :], bq[0:64, ct:ct + 1])
                    nc.vector.tensor_scalar_add(
                        qtp[ct][1][64:128, :], pq[64:128, :],
                        bq[64:128, ct:ct + 1])
                else:
                    nc.vector.tensor_scalar_add(
                        ktt[ct - ET], pq, bq[:, ct:ct + 1])

            # ---- pre-attention: v_aug (8 tiles) + all 12 q/k column tiles
            # through one 2-buffer psum tag (scoped pool; space reclaimed
            # after). v and qk interleave so the DVE work (one strided cast
            # per v tile, one bias-add per qk tile) stays off the MM rhythm.
            with tc.tile_pool(name="psA", bufs=2, space="PSUM") as psA:
                # exact 1.0 into the per-head ones columns (DVE in0*0 + 1;
                # strided memset is rejected by this walrus's ISA check) —
                # independent of the matmuls, emitted up front
                bq12 = bq[:, 0:12].rearrange("p (a b) -> p a b", b=1)
                for it in range(IT):
                    ones_cols = v_aug[it].rearrange(
                        "p (h c) -> p h c", c=128)[:, :, 64:65]
                    nc.vector.tensor_scalar(
                        ones_cols, bq12, 0.0, 1.0,
                        mybir.AluOpType.mult, mybir.AluOpType.add)

                def emit_v(it):
                    pvv = psA.tile([128, N], f32, tag="vq", name=f"pv_{it}")
                    for (n0, nw) in ((0, 512), (512, 256)):
                        for e in range(ET):
                            nc.tensor.matmul(
                                pvv[:, n0:n0 + nw],
                                xT[e][:, it * 128:(it + 1) * 128],
                                wv[e][:, n0:n0 + nw],
                                start=(e == 0), stop=(e == ET - 1))
                    # single strided cast: [128,768] f32 -> per-head 64-col
                    # groups of v_aug (stride 128)
                    nc.vector.tensor_copy(
                        out=v_aug[it].rearrange(
                            "p (h c) -> p h c", c=128)[:, :, 0:64],
                        in_=pvv[:, 0:E].rearrange("p (h c) -> p h c", c=64))

                def emit_qk(ct):
                    pq = psA.tile([128, N], f32, tag="vq", name=f"pq_{ct}")
                    for i in range(12):
                        emit_qk_mm(pq, ct, i)
                    emit_qk_bias(pq, ct)

                # attention pair hp needs cts (hp, 6+hp); order qk so pair 0
                # unblocks first, v tiles spread between
                qk_order = [0, ET, 1, ET + 1, 2, ET + 2, 3, ET + 3,
                            4, ET + 4, 5, ET + 5]
                emit_v(0)
                for z in range(8):
                    emit_qk(qk_order[z])
                    if z + 1 < IT:
                        emit_v(z + 1)
                for z in range(8, 12):
                    emit_qk(qk_order[z])

            # ---- attention: per head-pair; next pair's q/k projections
            # interleave as PE filler (3 MMs per jt slot)
            ovT = [pp.tile([128, N], bf16, tag=f"ovT{e}", name=f"ovT{e}")
                   for e in range(ET)]

            with tc.tile_pool(name="psB", bufs=1, space="PSUM") as psB:
                for hp in range(HP):
                    h0, h1 = 2 * hp, 2 * hp + 1
                    qt0, qt1 = qtp[hp][0], qtp[hp][1]
                    kt = ktt[hp]
                    pv0 = psB.tile([128, N], f32, tag="pv", bufs=2,
                                   name=f"pv0_{hp}")
                    pv1 = psB.tile([128, N], f32, tag="pv", bufs=2,
                                   name=f"pv1_{hp}")
                    for jt in range(IT):
                        js = slice(jt * 128, (jt + 1) * 128)
                        # S^T via K=128 against the full shared kt
                        # stationary; the zero-padded q halves select the
                        # head. Full-array MMs keep HAM at 2.4 GHz.
                        st0 = psB.tile([128, N], f32, tag="st", bufs=2,
                                       name=f"st0_{hp}_{jt}")
                        for ih in range(2):
                            isl = slice(ih * 512, (ih + 1) * 512)
                            nc.tensor.matmul(st0[:, isl], kt[:, js],
                                             qt0[:, isl])
                        pT0 = ppT.tile([128, N], bf16, tag="pT",
                                       name=f"pT0_{hp}_{jt}")
                        nc.scalar.activation(out=pT0, in_=st0, func=AF.Exp)
                        st1 = psB.tile([128, N], f32, tag="st", bufs=2,
                                       name=f"st1_{hp}_{jt}")
                        for ih in range(2):
                            isl = slice(ih * 512, (ih + 1) * 512)
                            nc.tensor.matmul(st1[:, isl], kt[:, js],
                                             qt1[:, isl])
                        pT1 = ppT.tile([128, N], bf16, tag="pT",
                                       name=f"pT1_{hp}_{jt}")
                        nc.scalar.activation(out=pT1, in_=st1, func=AF.Exp)
                        for ih in range(2):
                            isl = slice(ih * 512, (ih + 1) * 512)
                            nc.tensor.matmul(
                                pv0[:, isl],
                                v_aug[jt][:, h0 * 128:h0 * 128 + 128],
                                pT0[:, isl], start=(jt == 0),
                                stop=(jt == IT - 1))
                        for ih in range(2):
                            isl = slice(ih * 512, (ih + 1) * 512)
                            nc.tensor.matmul(
                                pv1[:, isl],
                                v_aug[jt][:, h1 * 128:h1 * 128 + 128],
                                pT1[:, isl], start=(jt == 0),
                                stop=(jt == IT - 1))

                    # normalization. First evict both accumulators to SBUF
                    # with two fast DVE copies — this frees the pv PSUM
                    # slots ~2us after the pair ends so the next pair's PV
                    # matmuls never stall (the full normalization chain is
                    # ~8us of serial DVE+DMA latency and would otherwise
                    # idle the PE long enough to re-throttle HAM every
                    # pair). Then: colsum rows round-trip through DRAM
                    # reshaped to [128,16] so the multi-cycle DVE reciprocal
                    # runs on 128 lanes (~0.3us, not 6.5us); DRAM-broadcast
                    # 1/s to 64 partitions (SBUF APs cannot have zero
                    # partition step, DRAM APs can); multiply. Odd head
                    # moved into its ovT partition range by DMA (DVE cannot
                    # cross partitions).
                    s0 = prb.tile([65, N], f32, tag="pvc", bufs=4,
                                  name=f"s0_{hp}")
                    s1 = prb.tile([65, N], f32, tag="pvc", bufs=4,
                                  name=f"s1_{hp}")
                    nc.vector.tensor_copy(out=s0, in_=pv0[0:65, :])
                    nc.vector.tensor_copy(out=s1, in_=pv1[0:65, :])
                    ds = pdram.tile([2, N], f32, tag="ds", name=f"ds_{hp}")
                    nc.sync.dma_start(out=ds[0:1, :], in_=s0[64:65, :])
                    nc.sync.dma_start(out=ds[1:2, :], in_=s1[64:65, :])
                    rsq = prb.tile([128, 16], f32, tag="rsq", name=f"rsq_{hp}")
                    nc.sync.dma_start(
                        out=rsq,
                        in_=bass.AP(tensor=ds.tensor, offset=ds.offset,
                                    ap=[[16, 128], [1, 16]]))
                    rrq = prb.tile([128, 16], f32, tag="rsq", name=f"rrq_{hp}")
                    nc.vector.reciprocal(out=rrq, in_=rsq)
                    dr = pdram.tile([2, N], f32, tag="dr", name=f"dr_{hp}")
                    nc.sync.dma_start(
                        out=bass.AP(tensor=dr.tensor, offset=dr.offset,
                                    ap=[[16, 128], [1, 16]]),
                        in_=rrq)
                    rb0 = prb.tile([64, N], f32, tag="nrm", name=f"rb0_{hp}")
                    rb1 = prb.tile([64, N], f32, tag="nrm", name=f"rb1_{hp}")
                    nc.sync.dma_start(
                        out=rb0,
                        in_=bass.AP(tensor=dr.tensor, offset=dr.offset,
                                    ap=[[0, 64], [1, N]]))
                    nc.sync.dma_start(
                        out=rb1,
                        in_=bass.AP(tensor=dr.tensor, offset=dr[1:2, :].offset,
                                    ap=[[0, 64], [1, N]]))
                    nc.vector.tensor_mul(ovT[hp][0:64, :], s0[0:64, :], rb0)
                    tmp1 = prb.tile([64, N], bf16, tag="nrm", name=f"tmp1_{hp}")
                    nc.vector.tensor_mul(tmp1, s1[0:64, :], rb1)
                    nc.sync.dma_start(out=ovT[hp][64:128, :], in_=tmp1)

                # ---- proj: y = ovT^T @ w_proj + b_out. Lives inside the
                # attention pool reusing the 'st' slots (no pool-boundary
                # drain). e=0..4 for an it-pair emit before their e=5 chunks
                # so only the last head-pair's ovT gates the tail.
                for g in range(IT // 2):
                    its = (2 * g, 2 * g + 1)
                    pyts = {}
                    for it in its:
                        isl = slice(it * 128, (it + 1) * 128)
                        pyt = psB.tile([128, E], f32, tag="st", bufs=2,
                                       name=f"py_{it}")
                        pyts[it] = pyt
                        for (n0, nw) in ((0, 512), (512, 256)):
                            for e in range(ET - 1):
                                nc.tensor.matmul(
                                    pyt[:, n0:n0 + nw],
                                    ovT[e][:, isl],
                                    wp[e][:, n0:n0 + nw],
                                    start=(e == 0), stop=False)
                    for it in its:
                        isl = slice(it * 128, (it + 1) * 128)
                        pyt = pyts[it]
                        for (n0, nw) in ((0, 512), (512, 256)):
                            nc.tensor.matmul(
                                pyt[:, n0:n0 + nw],
                                ovT[ET - 1][:, isl],
                                wp[ET - 1][:, n0:n0 + nw],
                                start=False, stop=True)
                        ysb = py.tile([128, E], f32, tag="y", name=f"y{it}")
                        nc.vector.tensor_add(ysb, pyt, bo)
                        nc.sync.dma_start(out=y_d[isl, :], in_=ysb)

    _split_excess_waits(nc)
    return nc


def _get_nc():
    if not _NC_CACHE:
        _NC_CACHE.append(_build_nc())
    return _NC_CACHE[0]


# ---------------------------------------------------------------- entry point
def kernel(x, w_qkv, b_qkv, w_proj, b_proj, _trace=False):
    from concourse.bass_utils import run_bass_kernel_spmd

    import ml_dtypes
    bf16 = ml_dtypes.bfloat16
    x = np.asarray(x)
    w_qk, b_qk, w_v, b_out = _prep_weights(
        np.asarray(w_qkv), np.asarray(b_qkv), np.asarray(w_proj),
        np.asarray(b_proj))
    w_qk16 = w_qk.astype(bf16)
    w_v16 = w_v.astype(bf16)
    w_proj16 = np.ascontiguousarray(np.asarray(w_proj)).astype(bf16)

    in_maps = []
    for b in range(B):
        in_maps.append({
            "xT": np.ascontiguousarray(x[b].T).astype(bf16),
            "w_qk": w_qk16,
            "b_qk": b_qk,
            "w_v": w_v16,
            "w_proj": w_proj16,
            "b_out": b_out,
        })

    nc = _get_nc()
    res = run_bass_kernel_spmd(nc, in_maps, core_ids=list(range(B)),
                               trace=_trace)
    out = np.stack([res.results[b]["y"] for b in range(B)]).astype(np.float32)
    if _trace:
        return out, res
    return out


# revision 20
# speedup vs baseline: 1.3343x; 1.2511x over previous
"""Trainium2 Bass kernel for nn_Attention_78554951844258.

Dense 12-head attention block: qkv = x@Wqkv+b; RoPE(q,k); softmax(q k^T/sqrt(d)) v; proj.

Sharding: data-parallel over batch — each of the 8 NeuronCores computes one
batch element end-to-end (no collectives).

Algebraic restructuring (host-side, exact, O(weights)):
  * The reference applies RoPE with seq_dim=1 on [b,h,n,d], so cos/sin depend
    only on (head, dim) — RoPE is a position-independent per-head 64x64 linear
    map M_h that folds into the q/k columns of w_qkv (and biases).
  * The softmax scale 1/sqrt(d) folds into the q weights.
  * The v bias and proj bias fold into a single output bias
    b_out = b_v @ w_proj + b_proj, because softmax rows sum to 1.
  * Softmax max-subtraction is skipped: folded scores are bounded (|S| < ~3),
    exp is safe in fp32 and the result is mathematically identical.

v2 schedule (HAM-aware): the v1 kernel ran ~360us because each head-pair's
normalization tail (6.5us single-partition DVE reciprocals feeding the next
pair's bias-adds through the in-order DVE queue) idled the PE >3.4us, HAM
re-throttled to 1.2GHz, and ~70% of the matmul stream ran at half clock.
v2 keeps the PE dense:
  * input DMAs ordered by consumption (xT, wv, wqk, biases, wp);
  * v_aug and the first q/k column-tiles pipeline through one 2-buffer PSUM
    tag before attention;
  * the remaining 10 q/k projections stream as PE filler inside the
    attention loop (their own 2-bank PSUM slot), hidden under the ACT-bound
    exp pipeline;
  * PSUM = pq(2 banks) + st(2) + pv(2x2) = 8 banks exactly;
  * normalization uses reciprocal_approx_fast (~1.3us, 18-bit) and gates
    nothing but the pair's own ovT; broadcast via DRAM round-trip DMA;
  * proj accumulates e=0..4 ahead of the last pair's e=5 chunks to shrink
    the tail.
Matmul operands are bf16; accumulation fp32 in PSUM.
"""
import numpy as np

NUM_HEADS = 12
E = 768
D = 64
B = 8
N = 1024
HALF = D // 2


def _ensure_axon_hooks():
    """The NTFF profile hook registry module may be missing in a fresh
    container; (re)create it so trace=True profiling degrades gracefully."""
    try:
        import antenv.axon_hooks  # noqa: F401
        return
    except ImportError:
        pass
    try:
        import antenv
        import os
        p = os.path.join(os.path.dirname(antenv.__file__), "axon_hooks.py")
        with open(p, "w") as f:
            f.write(
                "_hook = None\n\n"
                "def set_axon_ntff_profile_hook(hook):\n"
                "    global _hook\n    _hook = hook\n\n"
                "def get_axon_ntff_profile_hook():\n"
                "    return _hook\n")
    except Exception:
        pass


_ensure_axon_hooks()


# ---------------------------------------------------------------- host math
def _rope_matrix():
    """M[h, x, d]: rope(q)[x] = sum_d M[h, x, d] * q[d] (float64)."""
    inv_freq = 1.0 / (10000.0 ** (np.arange(0, D, 2, dtype=np.float64) / D))
    t = np.arange(NUM_HEADS, dtype=np.float64)
    emb = np.concatenate([t[:, None] * inv_freq[None, :]] * 2, axis=-1)  # [H, D]
    cos, sin = np.cos(emb), np.sin(emb)
    M = np.zeros((NUM_HEADS, D, D))
    for h in range(NUM_HEADS):
        for d in range(D):
            M[h, d, d] = cos[h, d]
            if d < HALF:
                M[h, d, d + HALF] = -sin[h, d]
            else:
                M[h, d, d - HALF] = sin[h, d]
    return M


def _prep_weights(w_qkv, b_qkv, w_proj, b_proj):
    w = w_qkv.astype(np.float64)
    b = b_qkv.astype(np.float64)
    M = _rope_matrix()
    scale = float(D) ** (-0.5)
    w_q = w[:, 0:E].reshape(E, NUM_HEADS, D)
    w_k = w[:, E:2 * E].reshape(E, NUM_HEADS, D)
    b_q = b[0:E].reshape(NUM_HEADS, D)
    b_k = b[E:2 * E].reshape(NUM_HEADS, D)
    w_q2 = np.einsum('ehd,hxd->ehx', w_q, M) * scale
    b_q2 = np.einsum('hd,hxd->hx', b_q, M) * scale
    w_k2 = np.einsum('ehd,hxd->ehx', w_k, M)
    b_k2 = np.einsum('hd,hxd->hx', b_k, M)
    w_qk = np.ascontiguousarray(
        np.concatenate([w_q2.reshape(E, E), w_k2.reshape(E, E)], axis=1),
        dtype=np.float32)                                     # [E, 2E]
    b_qk = np.concatenate([b_q2.reshape(E), b_k2.reshape(E)]).astype(np.float32)
    w_v = np.ascontiguousarray(w[:, 2 * E:3 * E], dtype=np.float32)
    b_out = (b[2 * E:3 * E] @ w_proj.astype(np.float64)
             + b_proj.astype(np.float64)).astype(np.float32)
    return w_qk, b_qk, w_v, b_out


# ---------------------------------------------------------------- waitfix
def _split_excess_waits(nc):
    """walrus in this container rejects >4 sync waits per instruction (and
    fewer on Drain/SP-NoOp paths). Split overflow waits onto preceding
    same-engine 1-wait NOPs — semantically identical (sequencer blocks in
    order)."""
    import concourse.mybir as mybir
    import bass_rust
    counter = [0]

    def make_nop(engine):
        counter[0] += 1
        nop = bass_rust.InstNoOp(name=f"I-waitfix-{counter[0]}", ins=[], outs=[])
        nop.engine = engine
        return nop

    for fn in nc.m.functions:
        for bb in fn.blocks:
            insts = bb.instructions
            out = []
            changed = False
            for inst in insts:
                si = inst.sync_info
                waits = list(si.on_wait) if si is not None else []
                tn = type(inst).__name__
                keep = 0 if tn == "InstDrain" else 1
                if len(waits) > keep:
                    for w in waits[:len(waits) - keep]:
                        nop = make_nop(inst.engine)
                        nop.sync_info = mybir.SyncInfo(on_wait=[w], on_update=[])
                        out.append(nop)
                    inst.sync_info = mybir.SyncInfo(
                        on_wait=waits[len(waits) - keep:],
                        on_update=list(si.on_update))
                    changed = True
                out.append(inst)
            if changed:
                bb.instructions = out


# ---------------------------------------------------------------- device IR
_NC_CACHE = []


def _build_nc():
    import concourse.bass as bass
    import concourse.mybir as mybir
    from concourse.tile import TileContext

    dt = mybir.dt
    f32 = dt.float32
    bf16 = dt.bfloat16
    AF = mybir.ActivationFunctionType

    nc = bass.Bass(target_bir_lowering=False)
    xT_d = nc.dram_tensor("xT", [E, N], bf16, kind="ExternalInput")
    wqk_d = nc.dram_tensor("w_qk", [E, 2 * E], bf16, kind="ExternalInput")
    bqk_d = nc.dram_tensor("b_qk", [2 * E], f32, kind="ExternalInput")
    wv_d = nc.dram_tensor("w_v", [E, E], bf16, kind="ExternalInput")
    wp_d = nc.dram_tensor("w_proj", [E, E], bf16, kind="ExternalInput")
    bo_d = nc.dram_tensor("b_out", [E], f32, kind="ExternalInput")
    y_d = nc.dram_tensor("y", [N, E], f32, kind="ExternalOutput")

    ET = E // 128          # 6 e-tiles
    IT = N // 128          # 8 i/j-tiles
    HP = NUM_HEADS // 2    # 6 head pairs

    with TileContext(nc) as tc:
        with (
            tc.tile_pool(name="stat", bufs=1) as p1,         # xT, w_qk, wv, wp
            tc.tile_pool(name="persist", bufs=1) as pp,      # v_aug, qkt, ovT, biases
            tc.tile_pool(name="pT", bufs=4) as ppT,          # exp'd scores
            tc.tile_pool(name="nrm", bufs=6) as prb,         # recip + broadcast
            tc.tile_pool(name="yout", bufs=2) as py,         # y staging
            tc.tile_pool(name="dscr", bufs=4, space="DRAM") as pdram,
        ):
            # ---- loads, ordered by first use
            xT = [p1.tile([128, N], bf16, tag=f"xT{e}", name=f"xT{e}")
                  for e in range(ET)]
            wv = [p1.tile([128, E], bf16, tag=f"wv{e}", name=f"wv{e}")
                  for e in range(ET)]
            wqk = [p1.tile([128, 2 * E], bf16, tag=f"wqk{e}", name=f"wqk{e}")
                   for e in range(ET)]
            wp = [p1.tile([128, E], bf16, tag=f"wp{e}", name=f"wp{e}")
                  for e in range(ET)]
            for e in range(ET):
                nc.sync.dma_start(out=xT[e], in_=xT_d[e * 128:(e + 1) * 128, :])
            for e in range(ET):
                nc.sync.dma_start(out=wv[e], in_=wv_d[e * 128:(e + 1) * 128, :])
            for e in range(ET):
                nc.sync.dma_start(out=wqk[e], in_=wqk_d[e * 128:(e + 1) * 128, :])
            bq = pp.tile([128, 12], f32, tag="bq")
            nc.sync.dma_start(out=bq, in_=bqk_d[:].rearrange("(t p) -> p t", p=128))
            bo = pp.tile([128, E], f32, tag="bo")
            nc.sync.dma_start(
                out=bo,
                in_=bass.AP(tensor=bo_d[:].tensor, offset=bo_d[:].offset,
                            ap=[[0, 128], [1, E]]))
            for e in range(ET):
                nc.sync.dma_start(out=wp[e], in_=wp_d[e * 128:(e + 1) * 128, :])

            # v_aug: per head 128 columns (64 v + ones + 63 zeros) so the PV
            # stationary is a full 128x128 load — HAM's activity monitor
            # only un-throttles the PE clock when the whole array is lit;
            # half-idle matmuls (M=65 / K=64) keep it at 1.2 GHz forever.
            v_aug = [pp.tile([128, NUM_HEADS * 128], bf16, tag=f"vaug{i}",
                             name=f"vaug{i}") for i in range(IT)]
            # q is stored zero-padded per head ([128, N] with the other
            # head's 64 partitions zeroed) so the S matmul contracts K=128
            # against the full shared kt stationary; k tiles stay packed.
            qtp = [[pp.tile([128, N], bf16, tag=f"qtp{c}_{h}",
                            name=f"qtp{c}_{h}") for h in range(2)]
                   for c in range(ET)]
            ktt = [pp.tile([128, N], bf16, tag=f"ktt{c}", name=f"ktt{c}")
                   for c in range(ET)]
            # zero-fills: no input deps — run on the otherwise-idle GpSimd
            # engine during the load phase (DVE is the pre-attention
            # bottleneck; 20 memsets there cost ~22us of queue delay)
            for it in range(IT):
                nc.gpsimd.memset(v_aug[it], 0.0)
            for c in range(ET):
                nc.gpsimd.memset(qtp[c][0][64:128, :], 0.0)
                nc.gpsimd.memset(qtp[c][1][0:64, :], 0.0)

            # q/k column-tile projection: 12 accumulating MMs + bias-add
            def emit_qk_mm(pq, ct, i):
                ih, e = divmod(i, ET)
                nc.tensor.matmul(
                    pq[:, ih * 512:(ih + 1) * 512],
                    wqk[e][:, ct * 128:(ct + 1) * 128],
                    xT[e][:, ih * 512:(ih + 1) * 512],
                    start=(e == 0), stop=(e == ET - 1))

            def emit_qk_bias(pq, ct):
                # bias-add on the (pre-attention-idle) ScalarE: Identity
                # activation with a per-partition bias AP. Keeps the DVE
                # queue free for the v casts.
                if ct < ET:
                    nc.scalar.activation(
                        out=qtp[ct][0][0:64, :], in_=pq[0:64, :],
                        func=AF.Identity, bias=bq[0:64, ct:ct + 1])
                    nc.scalar.activation(
                        out=qtp[ct][1][64:128, :], in_=pq[64:128, :],
                        func=AF.Identity, bias=bq[64:128, ct:ct + 1])
                else:
                    nc.scalar.activation(
                        out=ktt[ct - ET], in_=pq,
                        func=AF.Identity, bias=bq[:, ct:ct + 1])

            # ---- pre-attention: v_aug (8 tiles) + all 12 q/k column tiles
            # through one 2-buffer psum tag (scoped pool; space reclaimed
            # after). v and qk interleave so the DVE work (one strided cast
            # per v tile, one bias-add per qk tile) stays off the MM rhythm.
            with tc.tile_pool(name="psA", bufs=2, space="PSUM") as psA:
                # exact 1.0 into the per-head ones columns (DVE in0*0 + 1;
                # strided memset is rejected by this walrus's ISA check) —
                # independent of the matmuls, emitted up front
                bq12 = bq[:, 0:12].rearrange("p (a b) -> p a b", b=1)
                for it in range(IT):
                    ones_cols = v_aug[it].rearrange(
                        "p (h c) -> p h c", c=128)[:, :, 64:65]
                    nc.vector.tensor_scalar(
                        ones_cols, bq12, 0.0, 1.0,
                        mybir.AluOpType.mult, mybir.AluOpType.add)

                def emit_v(it):
                    pvv = psA.tile([128, N], f32, tag="vq", name=f"pv_{it}")
                    for (n0, nw) in ((0, 512), (512, 256)):
                        for e in range(ET):
                            nc.tensor.matmul(
                                pvv[:, n0:n0 + nw],
                                xT[e][:, it * 128:(it + 1) * 128],
                                wv[e][:, n0:n0 + nw],
                                start=(e == 0), stop=(e == ET - 1))
                    # single strided cast: [128,768] f32 -> per-head 64-col
                    # groups of v_aug (stride 128)
                    nc.vector.tensor_copy(
                        out=v_aug[it].rearrange(
                            "p (h c) -> p h c", c=128)[:, :, 0:64],
                        in_=pvv[:, 0:E].rearrange("p (h c) -> p h c", c=64))

                def emit_qk(ct):
                    pq = psA.tile([128, N], f32, tag="vq", name=f"pq_{ct}")
                    for i in range(12):
                        emit_qk_mm(pq, ct, i)
                    emit_qk_bias(pq, ct)

                # attention pair hp needs cts (hp, 6+hp); order qk so pair 0
                # unblocks first, v tiles spread between
                qk_order = [0, ET, 1, ET + 1, 2, ET + 2, 3, ET + 3,
                            4, ET + 4, 5, ET + 5]
                emit_v(0)
                for z in range(8):
                    emit_qk(qk_order[z])
                    if z + 1 < IT:
                        emit_v(z + 1)
                for z in range(8, 12):
                    emit_qk(qk_order[z])

            # ---- attention: per head-pair; next pair's q/k projections
            # interleave as PE filler (3 MMs per jt slot)
            ovT = [pp.tile([128, N], bf16, tag=f"ovT{e}", name=f"ovT{e}")
                   for e in range(ET)]

            with tc.tile_pool(name="psB", bufs=1, space="PSUM") as psB:
                for hp in range(HP):
                    h0, h1 = 2 * hp, 2 * hp + 1
                    qt0, qt1 = qtp[hp][0], qtp[hp][1]
                    kt = ktt[hp]
                    pv0 = psB.tile([128, N], f32, tag="pv", bufs=2,
                                   name=f"pv0_{hp}")
                    pv1 = psB.tile([128, N], f32, tag="pv", bufs=2,
                                   name=f"pv1_{hp}")
                    for jt in range(IT):
                        js = slice(jt * 128, (jt + 1) * 128)
                        # S^T via K=128 against the full shared kt
                        # stationary; the zero-padded q halves select the
                        # head. Full-array MMs keep HAM at 2.4 GHz.
                        st0 = psB.tile([128, N], f32, tag="st", bufs=2,
                                       name=f"st0_{hp}_{jt}")
                        for ih in range(2):
                            isl = slice(ih * 512, (ih + 1) * 512)
                            nc.tensor.matmul(st0[:, isl], kt[:, js],
                                             qt0[:, isl])
                        pT0 = ppT.tile([128, N], bf16, tag="pT",
                                       name=f"pT0_{hp}_{jt}")
                        nc.scalar.activation(out=pT0, in_=st0, func=AF.Exp)
                        st1 = psB.tile([128, N], f32, tag="st", bufs=2,
                                       name=f"st1_{hp}_{jt}")
                        for ih in range(2):
                            isl = slice(ih * 512, (ih + 1) * 512)
                            nc.tensor.matmul(st1[:, isl], kt[:, js],
                                             qt1[:, isl])
                        pT1 = ppT.tile([128, N], bf16, tag="pT",
                                       name=f"pT1_{hp}_{jt}")
                        nc.scalar.activation(out=pT1, in_=st1, func=AF.Exp)
                        for ih in range(2):
                            isl = slice(ih * 512, (ih + 1) * 512)
                            nc.tensor.matmul(
                                pv0[:, isl],
                                v_aug[jt][:, h0 * 128:h0 * 128 + 128],
                                pT0[:, isl], start=(jt == 0),
                                stop=(jt == IT - 1))
                        for ih in range(2):
                            isl = slice(ih * 512, (ih + 1) * 512)
                            nc.tensor.matmul(
                                pv1[:, isl],
                                v_aug[jt][:, h1 * 128:h1 * 128 + 128],
                                pT1[:, isl], start=(jt == 0),
                                stop=(jt == IT - 1))

                    # normalization. First evict both accumulators to SBUF
                    # with two fast DVE copies — this frees the pv PSUM
                    # slots ~2us after the pair ends so the next pair's PV
                    # matmuls never stall (the full normalization chain is
                    # ~8us of serial DVE+DMA latency and would otherwise
                    # idle the PE long enough to re-throttle HAM every
                    # pair). Then: colsum rows round-trip through DRAM
                    # reshaped to [128,16] so the multi-cycle DVE reciprocal
                    # runs on 128 lanes (~0.3us, not 6.5us); DRAM-broadcast
                    # 1/s to 64 partitions (SBUF APs cannot have zero
                    # partition step, DRAM APs can); multiply. Odd head
                    # moved into its ovT partition range by DMA (DVE cannot
                    # cross partitions).
                    s0 = prb.tile([65, N], f32, tag="pvc", bufs=4,
                                  name=f"s0_{hp}")
                    s1 = prb.tile([65, N], f32, tag="pvc", bufs=4,
                                  name=f"s1_{hp}")
                    nc.vector.tensor_copy(out=s0, in_=pv0[0:65, :])
                    nc.vector.tensor_copy(out=s1, in_=pv1[0:65, :])
                    ds = pdram.tile([2, N], f32, tag="ds", name=f"ds_{hp}")
                    nc.sync.dma_start(out=ds[0:1, :], in_=s0[64:65, :])
                    nc.sync.dma_start(out=ds[1:2, :], in_=s1[64:65, :])
                    rsq = prb.tile([128, 16], f32, tag="rsq", name=f"rsq_{hp}")
                    nc.sync.dma_start(
                        out=rsq,
                        in_=bass.AP(tensor=ds.tensor, offset=ds.offset,
                                    ap=[[16, 128], [1, 16]]))
                    rrq = prb.tile([128, 16], f32, tag="rsq", name=f"rrq_{hp}")
                    nc.vector.reciprocal(out=rrq, in_=rsq)
                    dr = pdram.tile([2, N], f32, tag="dr", name=f"dr_{hp}")
                    nc.sync.dma_start(
                        out=bass.AP(tensor=dr.tensor, offset=dr.offset,
                                    ap=[[16, 128], [1, 16]]),
                        in_=rrq)
                    rb0 = prb.tile([64, N], f32, tag="nrm", name=f"rb0_{hp}")
                    rb1 = prb.tile([64, N], f32, tag="nrm", name=f"rb1_{hp}")
                    nc.sync.dma_start(
                        out=rb0,
                        in_=bass.AP(tensor=dr.tensor, offset=dr.offset,
                                    ap=[[0, 64], [1, N]]))
                    nc.sync.dma_start(
                        out=rb1,
                        in_=bass.AP(tensor=dr.tensor, offset=dr[1:2, :].offset,
                                    ap=[[0, 64], [1, N]]))
                    nc.vector.tensor_mul(ovT[hp][0:64, :], s0[0:64, :], rb0)
                    tmp1 = prb.tile([64, N], bf16, tag="nrm", name=f"tmp1_{hp}")
                    nc.vector.tensor_mul(tmp1, s1[0:64, :], rb1)
                    nc.sync.dma_start(out=ovT[hp][64:128, :], in_=tmp1)

                # ---- proj: y = ovT^T @ w_proj + b_out. Lives inside the
                # attention pool reusing the 'st' slots (no pool-boundary
                # drain). e=0..4 for an it-pair emit before their e=5 chunks
                # so only the last head-pair's ovT gates the tail.
                for g in range(IT // 2):
                    its = (2 * g, 2 * g + 1)
                    pyts = {}
                    for it in its:
                        isl = slice(it * 128, (it + 1) * 128)
                        pyt = psB.tile([128, E], f32, tag="st", bufs=2,
                                       name=f"py_{it}")
                        pyts[it] = pyt
                        for (n0, nw) in ((0, 512), (512, 256)):
                            for e in range(ET - 1):
                                nc.tensor.matmul(
                                    pyt[:, n0:n0 + nw],
                                    ovT[e][:, isl],
                                    wp[e][:, n0:n0 + nw],
                                    start=(e == 0), stop=False)
                    for it in its:
                        isl = slice(it * 128, (it + 1) * 128)
                        pyt = pyts[it]
                        for (n0, nw) in ((0, 512), (512, 256)):
                            nc.tensor.matmul(
                                pyt[:, n0:n0 + nw],
                                ovT[ET - 1][:, isl],
                                wp[ET - 1][:, n0:n0 + nw],
                                start=False, stop=True)
                        ysb = py.tile([128, E], f32, tag="y", name=f"y{it}")
                        nc.vector.tensor_add(ysb, pyt, bo)
                        nc.sync.dma_start(out=y_d[isl, :], in_=ysb)

    _split_excess_waits(nc)
    return nc


def _get_nc():
    if not _NC_CACHE:
        _NC_CACHE.append(_build_nc())
    return _NC_CACHE[0]


# ---------------------------------------------------------------- entry point
def kernel(x, w_qkv, b_qkv, w_proj, b_proj, _trace=False):
    from concourse.bass_utils import run_bass_kernel_spmd

    import ml_dtypes
    bf16 = ml_dtypes.bfloat16
    x = np.asarray(x)
    w_qk, b_qk, w_v, b_out = _prep_weights(
        np.asarray(w_qkv), np.asarray(b_qkv), np.asarray(w_proj),
        np.asarray(b_proj))
    w_qk16 = w_qk.astype(bf16)
    w_v16 = w_v.astype(bf16)
    w_proj16 = np.ascontiguousarray(np.asarray(w_proj)).astype(bf16)

    in_maps = []
    for b in range(B):
        in_maps.append({
            "xT": np.ascontiguousarray(x[b].T).astype(bf16),
            "w_qk": w_qk16,
            "b_qk": b_qk,
            "w_v": w_v16,
            "w_proj": w_proj16,
            "b_out": b_out,
        })

    nc = _get_nc()
    res = run_bass_kernel_spmd(nc, in_maps, core_ids=list(range(B)),
                               trace=_trace)
    out = np.stack([res.results[b]["y"] for b in range(B)]).astype(np.float32)
    if _trace:
        return out, res
    return out
